# revision 39
# baseline (speedup 1.0000x reference)
"""Trainium2 8-core Bass kernel for a causal multi-head attention block.

Module: qkv = x @ w_qkv + b_qkv ; causal MHA (16 heads, hd=64) ; out = attn @ w_out + b_out
Shapes: x (4, 2048, 1024) f32 -> out (4, 2048, 1024) f32.

Sharding (8 NeuronCores): tensor-parallel over heads - core c owns heads
{2c, 2c+1} for ALL batches (QKV weight columns sharded head-wise). After
attention, four progressive 8-way AllToAlls (one per 512-query chunk j)
convert head-sharding to token-sharding: in AllToAll j, core c receives the
256-token chunk (batch c//2, queries [512j+256*(c%2), +256)). Each core runs
the out projection for its four 256-token chunks as the A2As land and writes
its (1024, 1024) f32 output slice; the host reassembles the full output.

Device algorithm per core (bf16 matmul operands, f32 PSUM accumulation):
  1. Q^T, K^T (head-cols on partitions) and V (tokens on partitions, with a
     constant ones-column appended at col 65) from streamed x^T tiles
  2. scores S^T[k,q] = K^T.T @ Q^T per head (K=64; two heads via PE row
     tiling on partitions 0-63 / 64-127, which run CONCURRENTLY); exp on
     ScalarE with the 1/8 softmax scale folded in; causal masking via gpsimd
     affine_select on diagonal blocks
  3. PV in [q, d] orientation: out[q, d] accumulates per 128-query chunk qc
     with lhsT = P^T[k, qc-block] and rhs = V[k, 65].  Matmul cost on the PE
     is output-free-size (65) per instruction, so this form costs ~520 rows
     per key block instead of 1024 in the [d, q] form (LDWEIGHTS of the
     128x128 stationary hides under the previous matmul; measured 36ns
     median issue spacing).  The ones-column of V makes the softmax
     denominator l land in PSUM column 64 per q-partition.
     PSUM has_written semantics: start=True clears the accumulate bits for
     the WHOLE bank, so only the chunk's very first PV matmul per head-bank
     carries start=True; the other qc slots first-write with start=False
     (bits clear after the bank-wide reset -> overwrite, then accumulate).
  4. chunk epilogue AFTER the last PV block (a PSUM bank must not be read
     while the PE can still write it - bank collisions are fatal):
     per-head reciprocal of the l column (one DVE op spanning all qc slots,
     which also carries the RAW dep on the final PV write), then per-qc
     tensor_scalar_mul into a [128 q, 128 feat] bf16 tile DMAd straight into
     the AllToAll buffer in [token, feature] layout.  The RECEIVER side
     loads a2a_out with dma_start_transpose (DMA-crossbar transpose, off all
     compute engines) to get the [feature, token] lhsT the out projection
     needs.
  5. attention runs j-major so AllToAll j fires at the earliest point; a
     dummy warmup collective absorbs CC init; out-projection chunks 0/1 run
     as PE filler in the exp-bound late quarters, chunk 2 is reserved (via
     tile_wait_until scheduler hints, sim-time ~ 0.85*HW-time) to fill the
     AllToAll-3 window, chunk 3 tails.  a2a_out loads are issued from the SP
     queue with hints so their CC-completion waits cannot head-of-line block
     a busy queue.
  6. QKV work units are fed one-at-a-time between attention blocks so the PE
     never starves while ScalarE chews the exps (keeps the PE p-state ramped)
"""

import os
import numpy as np
import ml_dtypes

B, N, C, H, HD = 4, 2048, 1024, 16, 64
SCALE = HD ** -0.5
P = 128
CB = C // P               # 8 contraction blocks
NQC = N // 512            # 4 query chunks per batch
TOK_G = 1024              # tokens per core after A2A (output slice rows)
NHC = 2                   # heads per core
VC = NHC * HD             # 128 v columns per core

BF16 = ml_dtypes.bfloat16

_CACHE = {}


def _build_nc():
    import concourse.bass as bass
    import concourse.tile as tile
    from concourse import bacc, mybir
    from concourse.bass import ts, ds
    from contextlib import ExitStack

    FP = mybir.dt.float32
    BF = mybir.dt.bfloat16
    EXP = mybir.ActivationFunctionType.Exp

    nc = bacc.Bacc(num_devices=8)

    # per-core inputs
    xT_p = nc.declare_dram_parameter("xT", [B, C, N], BF, isOutput=False)
    wqk_p = nc.declare_dram_parameter("wqk", [C, 2 * VC], BF, isOutput=False)
    wv_p = nc.declare_dram_parameter("wv", [C, VC], BF, isOutput=False)
    wout_p = nc.declare_dram_parameter("wout", [C, C], BF, isOutput=False)
    bqk_p = nc.declare_dram_parameter("bqk", [P, 2], FP, isOutput=False)
    bv_p = nc.declare_dram_parameter("bv", [1, VC], BF, isOutput=False)
    bout_p = nc.declare_dram_parameter("bout", [1, C], BF, isOutput=False)
    cmask_p = nc.declare_dram_parameter("cmask", [P, 2, P], BF, isOutput=False)
    out_p = nc.declare_dram_parameter("out", [TOK_G, C], FP, isOutput=True)

    # A2A j carries, for target core s, the 256-token chunk
    # (batch s//2, queries [512j + 256*(s%2), +256)) in [token, feature]
    # layout; the receiver transposes on load via the DMA crossbar.
    a2a_in = [nc.dram_tensor(f"a2a_in{j}", [8, 256, P], BF) for j in range(NQC)]
    a2a_out = [nc.dram_tensor(f"a2a_out{j}", [8, 256, P], BF) for j in range(NQC)]
    warm_in = nc.dram_tensor("warm_in", [8, 1, 16], BF)
    warm_out = nc.dram_tensor("warm_out", [8, 1, 16], BF)

    def all_to_all(ins, outs):
        nc.gpsimd.collective_compute(
            "AllToAll",
            mybir.AluOpType.bypass,
            replica_groups=[list(range(8))],
            ins=[ins[:].opt()],
            outs=[outs[:].opt()],
        )

    with tile.TileContext(nc) as tc, ExitStack() as ctx:
        # dummy collective first: absorbs CC rendezvous + warmup latency
        all_to_all(warm_in, warm_out)

        const = ctx.enter_context(tc.tile_pool(name="const", bufs=1))
        big = ctx.enter_context(tc.tile_pool(name="big", bufs=1))
        xt_pool = ctx.enter_context(tc.tile_pool(name="xtp", bufs=4))
        pt_pool = ctx.enter_context(tc.tile_pool(name="ptp", bufs=5))
        # PSUM budget (16KB/partition): ps 3x[128,2,512]f32 = 12KB,
        # po 2x[128,4,128]f32 = 4KB (one bank per head, single chunk in
        # flight - the epilogue reads finish ~2us before the next chunk's
        # first PV needs the banks, and the deeper ps ring lets scores run
        # 3 blocks ahead of the exp stream instead of 2).
        ps_pool = ctx.enter_context(tc.tile_pool(name="psp", bufs=3, space="PSUM"))
        po_pool = ctx.enter_context(tc.tile_pool(name="pop", bufs=2, space="PSUM"))
        misc = ctx.enter_context(tc.tile_pool(name="misc", bufs=4))
        outp = ctx.enter_context(tc.tile_pool(name="outp", bufs=4))

        # first x tile and Q/K weights split in halves so the transfers run
        # in parallel across DMA queues and the PE can start ASAP
        xt_cache = {}

        def xt_fetch(b, tch, split=False):
            xt = xt_cache.get((b, tch))
            if xt is None:
                xt = xt_pool.tile([P, CB, 512], BF, tag="xt", name=f"xt{b}_{tch}")
                src = xT_p[b, :, ts(tch, 512)]
                if split:
                    nc.sync.dma_start(
                        xt[:, 0 : CB // 2, :],
                        src[0 : C // 2].rearrange("(cb p) t -> p cb t", p=P),
                    )
                    nc.sync.dma_start(
                        xt[:, CB // 2 : CB, :],
                        src[C // 2 : C].rearrange("(cb p) t -> p cb t", p=P),
                    )
                else:
                    nc.sync.dma_start(xt, src.rearrange("(cb p) t -> p cb t", p=P))
                xt_cache[(b, tch)] = xt
            return xt

        # first-half weights + first-half x first: the opening Q-chain's
        # kb=0..3 LDW/matmuls only touch those sub-regions, so the PE can
        # start while the second halves are still in flight
        wqk = const.tile([P, CB, 2 * VC], BF)
        nc.sync.dma_start(
            wqk[:, 0 : CB // 2, :],
            wqk_p[0 : C // 2].rearrange("(cb p) c -> p cb c", p=P),
        )
        xt_fetch(0, 0, split=True)
        nc.sync.dma_start(
            wqk[:, CB // 2 : CB, :],
            wqk_p[C // 2 : C].rearrange("(cb p) c -> p cb c", p=P),
        )
        wv = const.tile([P, CB, VC], BF)
        nc.sync.dma_start(wv, wv_p.rearrange("(cb p) c -> p cb c", p=P))
        bqk = const.tile([P, 2], FP)
        nc.sync.dma_start(bqk, bqk_p[:])
        bv = const.tile([1, VC], BF)
        nc.sync.dma_start(bv, bv_p[:])
        cmask = const.tile([P, 2, P], BF)
        nc.sync.dma_start(cmask, cmask_p[:])
        bvb = const.tile([P, VC], BF)
        nc.gpsimd.partition_broadcast(bvb, bv)

        # out-projection weights/bias tiles (DMA deferred past startup so the
        # 2MB transfer does not delay the first x-tile loads)
        wout = big.tile([P, CB, C], BF)
        bout = const.tile([1, C], BF)
        boutb = const.tile([P, C], BF)

        def load_wout():
            nc.sync.dma_start(wout, wout_p.rearrange("(cb p) c -> p cb c", p=P))
            nc.sync.dma_start(bout, bout_p[:])
            nc.gpsimd.partition_broadcast(boutb, bout)

        # per-(batch, 512-token-chunk) Q^T/K^T and V tiles for fine deps
        qk_t = [
            [big.tile([P, 2, 512], BF, name=f"qkT{b}_{t}") for t in range(NQC)]
            for b in range(B)
        ]
        v_t = []
        for b in range(B):
            row = []
            for t in range(NQC):
                vt = big.tile([P, 4, NHC, HD + 1], BF, name=f"v{b}_{t}")
                nc.vector.memset(vt[:, :, :, HD : HD + 1], 1.0)
                row.append(vt)
            v_t.append(row)

        # ---- QKV work units ----------------------------------------------
        # unit order per group (b,t): [Q, K, V0, V1, V2, V3]; every unit
        # closes its own PSUM tile so pool rotation can never deadlock the
        # in-order PE behind a later-emitted consumer
        def prefetch_ahead():
            # keep up to 3 not-yet-consumed x tiles in flight so no group's
            # first matmul ever waits on its 1MB DMA (a one-ahead prefetch
            # misses the demand jumps at phase transitions: 19us stall)
            pending = sum(
                1 for g in feed_seq if g in xt_cache and emitted[g] == 0
            )
            if pending >= 3:
                return
            for g in feed_seq:
                if emitted[g] < 6 and g not in xt_cache:
                    xt_fetch(*g)
                    return

        def qkv_unit(b, tch, u):
            xt = xt_fetch(b, tch)
            if u == 0:
                prefetch_ahead()
            if u < 2:
                qk = u
                psq = ps_pool.tile([P, 512], FP, tag="ps", name=f"psq{b}_{tch}_{qk}")
                for kb in range(CB):
                    nc.tensor.matmul(
                        psq,
                        lhsT=wqk[:, kb, ts(qk, P)],
                        rhs=xt[:, kb, :],
                        start=(kb == 0),
                        stop=(kb == CB - 1),
                        skip_group_check=True,
                    )
                nc.vector.tensor_scalar_add(
                    qk_t[b][tch][:, qk, :], psq, bqk[:, qk : qk + 1]
                )
            else:
                tb4 = u - 2
                psv = ps_pool.tile([P, VC], FP, tag="ps", name=f"psv{b}_{tch}_{tb4}")
                for kb in range(CB):
                    nc.tensor.matmul(
                        psv,
                        lhsT=xt[:, kb, ts(tb4, P)],
                        rhs=wv[:, kb, :],
                        start=(kb == 0),
                        stop=(kb == CB - 1),
                        skip_group_check=True,
                    )
                nc.vector.tensor_tensor(
                    v_t[b][tch][:, tb4, :, 0:HD],
                    psv.rearrange("p (h d) -> p h d", h=NHC),
                    bvb.rearrange("p (h d) -> p h d", h=NHC),
                    mybir.AluOpType.add,
                )

        # feed machinery: units emitted on demand (deps) or popped as filler
        emitted = {(b, t): 0 for b in range(B) for t in range(NQC)}
        feed_seq = [(b, t) for t in range(NQC) for b in range(B)]

        def emit_group(b, t, upto):
            while emitted[(b, t)] < upto:
                qkv_unit(b, t, emitted[(b, t)])
                emitted[(b, t)] += 1

        def pop_units(n, tmax=NQC):
            while n > 0:
                prefetch_ahead()
                for g in feed_seq:
                    if emitted[g] < 6:
                        qkv_unit(g[0], g[1], emitted[g])
                        emitted[g] += 1
                        break
                else:
                    return
                n -= 1

        # ---- attention ----------------------------------------------------
        def emit_attention(b, j):
            emit_group(b, j, 2)  # biased Q (and K) of chunk j
            # per-head PV accumulators: one PSUM bank each;
            # [q-in-chunk partitions, qc, 65 used of 128]
            po = [
                po_pool.tile([P, 4, P], FP, tag="po", name=f"po{b}_{j}_{hh}")
                for hh in range(2)
            ]
            nkb = 4 * j + 4
            pts = {}

            def emit_scores(i):
                m = max(0, i - 4 * j)
                w = 512 - P * m
                emit_group(b, i // 4, 2)           # K of this key chunk
                emit_group(b, i // 4, 3 + i % 4)   # V block for this i
                i2 = min(i + 3, nkb - 1)           # V three blocks ahead so
                emit_group(b, i2 // 4, 3 + i2 % 4)  # the PV never waits the DVE
                pss = ps_pool.tile([P, 2, 512], FP, tag="ps", name=f"pss{b}_{j}_{i}")
                for hh in range(2):
                    rlo = 64 * hh
                    nc.tensor.matmul(
                        pss[:, hh, P * m : 512],
                        lhsT=qk_t[b][i // 4][:, 1, :][rlo : rlo + 64, ts(i % 4, P)],
                        rhs=qk_t[b][j][:, 0, :][rlo : rlo + 64, ds(P * m, w)],
                        start=True,
                        stop=True,
                        skip_group_check=True,
                    )
                pt = pt_pool.tile([P, 2, 512], BF, tag="pt", name=f"pt{b}_{j}_{i}")
                nc.scalar.activation(
                    pt[:, :, P * m : 512], pss[:, :, P * m : 512], EXP, scale=SCALE
                )
                if i >= 4 * j:
                    # causal mask for the diagonal 128x128 sub-block: a
                    # constant upper-triangular multiply on the (idle) DVE -
                    # much lower latency in the exp->mask->PV chain than the
                    # gpsimd affine_select it replaces
                    nc.vector.tensor_tensor(
                        pt[:, :, P * m : P * m + P],
                        pt[:, :, P * m : P * m + P],
                        cmask,
                        mybir.AluOpType.mult,
                    )
                pts[i] = pt

            def emit_pv(i):
                m = max(0, i - 4 * j)
                pt = pts.pop(i)
                for hh in range(2):
                    for qc in range(m, 4):
                        nc.tensor.matmul(
                            po[hh][:, qc, 0 : HD + 1],
                            lhsT=pt[:, hh, ts(qc, P)],
                            rhs=v_t[b][i // 4][:, i % 4, hh, :],
                            start=(i == 0 and qc == 0),
                            stop=(i == 4 * j + qc),
                            skip_group_check=True,
                        )

            # 3-block software pipeline skew with filler popped before the
            # PV so exp(i)/mask(i) are long done when PV(i) issues
            for i in range(nkb):
                emit_scores(i)
                if i >= 3:
                    emit_pv(i - 3)
                pop_units(1)
            emit_pv(nkb - 3)
            emit_pv(nkb - 2)
            emit_pv(nkb - 1)

            # epilogue (after ALL PV writes into the po banks): normalize by
            # the l rider column and stage into the A2A buffer [tok, feat].
            # The rec read spans all qc slots so it carries the dep on the
            # bank's final PV write; the muls follow in DVE program order.
            rec = misc.tile([P, 2, 4, 1], FP, tag="rec")
            for hh in range(2):
                nc.vector.reciprocal_approx_fast(
                    rec[:, hh], po[hh][:, :, HD : HD + 1]
                )
            for qc in range(4):
                at_qd = misc.tile([P, 2, HD], BF, tag="atqd")
                for hh in range(2):
                    nc.vector.tensor_scalar_mul(
                        at_qd[:, hh, :], po[hh][:, qc, 0:HD], rec[:, hh, qc, :]
                    )
                nc.sync.dma_start(
                    a2a_in[j][2 * b + qc // 2][ds(P * (qc % 2), P), :], at_qd
                )

        # ---- out projection for one 256-token chunk ----------------------
        at_all = [[None] * CB for _ in range(NQC)]
        # tile_wait_until hints (scheduler-sim ms ~ 0.85e-3 * HW ns): keep
        # the CC-gated loads from being scheduled ahead of satisfiable work
        load_hint = {0: 0.120, 1: 0.190, 2: 0.245, 3: 0.290}

        def outproj_load(k):
            with tc.tile_wait_until(load_hint[k]):
                for kb in range(CB):
                    t = big.tile([P, 256], BF, name=f"at_all{k}_{kb}")
                    nc.sync.dma_start_transpose(t, a2a_out[k][kb])
                    at_all[k][kb] = t

        def outproj_chain(k, tb, co):
            if at_all[k][0] is None:
                outproj_load(k)
            py = ps_pool.tile([P, 512], FP, tag="ps")
            for kb in range(CB):
                nc.tensor.matmul(
                    py,
                    lhsT=at_all[k][kb][:, ts(tb, P)],
                    rhs=wout[:, kb, ts(co, 512)],
                    start=(kb == 0),
                    stop=(kb == CB - 1),
                    skip_group_check=True,
                )
            ot = outp.tile([P, 512], FP, tag="ot")
            nc.vector.tensor_add(ot, py, boutb[:, ts(co, 512)])
            nc.sync.dma_start(out_p[ds(256 * k + P * tb, P), ts(co, 512)], ot)

        def outproj_chunk(k, chain_hint=None):
            if at_all[k][0] is None:
                outproj_load(k)
            from contextlib import nullcontext

            cm = tc.tile_wait_until(chain_hint) if chain_hint else nullcontext()
            with cm:
                for tb in range(2):
                    for co in range(2):
                        outproj_chain(k, tb, co)

        # ---- main schedule ------------------------------------------------
        # out-projection chunks 0/1 double as PE filler for the exp-bound
        # late quarters; chunk 2 is held back (hint) to fill the AllToAll-3
        # window; chunk 3 tails.
        placements = {(2, 0): 0, (3, 0): 1}
        for j in range(NQC):
            for b in range(B):
                emit_attention(b, j)
                k = placements.get((j, b))
                if k is not None:
                    outproj_chunk(k)
            all_to_all(a2a_in[j], a2a_out[j])
            if j == 0:
                load_wout()
        pop_units(9999)
        outproj_chunk(2, chain_hint=0.293)
        outproj_chunk(NQC - 1)

    nc.finalize()
    return nc


def _get_nc():
    if "nc" not in _CACHE:
        _CACHE["nc"] = _build_nc()
    return _CACHE["nc"]


def _shard_inputs(x, w_qkv, b_qkv, w_out, b_out):
    x = np.asarray(x, dtype=np.float32)
    w_qkv = np.asarray(w_qkv, dtype=np.float32)
    b_qkv = np.asarray(b_qkv, dtype=np.float32)
    w_out = np.asarray(w_out, dtype=np.float32)
    b_out = np.asarray(b_out, dtype=np.float32)

    xT = np.ascontiguousarray(x.transpose(0, 2, 1)).astype(BF16)  # (B, C, N)
    wout_b = np.ascontiguousarray(w_out).astype(BF16)
    bout_r = np.ascontiguousarray(b_out[None, :]).astype(BF16)
    # cmask[k, hh, q] = 1 where q >= k (causal-valid within a diagonal
    # 128x128 block), replicated for both heads
    tri = np.triu(np.ones((P, P), dtype=np.float32)).astype(BF16)
    cmask = np.ascontiguousarray(np.repeat(tri[:, None, :], 2, axis=1))

    in_maps = []
    for c in range(8):
        c0 = HD * NHC * c  # first head-col owned by this core
        wq = w_qkv[:, 0 * C + c0 : 0 * C + c0 + VC]
        wk = w_qkv[:, 1 * C + c0 : 1 * C + c0 + VC]
        wvv = w_qkv[:, 2 * C + c0 : 2 * C + c0 + VC]
        bq = b_qkv[0 * C + c0 : 0 * C + c0 + VC]
        bk = b_qkv[1 * C + c0 : 1 * C + c0 + VC]
        bvv = b_qkv[2 * C + c0 : 2 * C + c0 + VC]
        in_maps.append(
            dict(
                xT=xT,
                wqk=np.ascontiguousarray(np.concatenate([wq, wk], axis=1)).astype(BF16),
                wv=np.ascontiguousarray(wvv).astype(BF16),
                wout=wout_b,
                bqk=np.ascontiguousarray(np.stack([bq, bk], axis=1)).astype(np.float32),
                bv=np.ascontiguousarray(bvv[None, :]).astype(BF16),
                bout=bout_r,
                cmask=cmask,
            )
        )
    return in_maps


def kernel(x, attention_mask, w_qkv, b_qkv, w_out, b_out):
    from concourse.bass_utils import run_bass_kernel_spmd

    nc = _get_nc()
    in_maps = _shard_inputs(x, w_qkv, b_qkv, w_out, b_out)
    res = run_bass_kernel_spmd(nc, in_maps, core_ids=list(range(8)))
    _CACHE["last_results"] = res
    out = np.empty((B, N, C), np.float32)
    for c in range(8):
        b = c // 2
        h = c % 2
        r = np.asarray(res.results[c]["out"]).astype(np.float32)
        for j in range(NQC):
            t0 = 512 * j + 256 * h
            out[b, t0 : t0 + 256] = r[256 * j : 256 * j + 256]
    return out


# revision 41
# speedup vs baseline: 1.0248x; 1.0248x over previous
"""Trainium2 8-core Bass kernel for a causal multi-head attention block.

Module: qkv = x @ w_qkv + b_qkv ; causal MHA (16 heads, hd=64) ; out = attn @ w_out + b_out
Shapes: x (4, 2048, 1024) f32 -> out (4, 2048, 1024) f32.

Sharding (8 NeuronCores): tensor-parallel over heads - core c owns heads
{2c, 2c+1} for ALL batches (QKV weight columns sharded head-wise). After
attention, four progressive 8-way AllToAlls (one per 512-query chunk j)
convert head-sharding to token-sharding: in AllToAll j, core c receives the
256-token chunk (batch c//2, queries [512j+256*(c%2), +256)). Each core runs
the out projection for its four 256-token chunks as the A2As land and writes
its (1024, 1024) f32 output slice; the host reassembles the full output.

Device algorithm per core (bf16 matmul operands, f32 PSUM accumulation):
  1. Q^T, K^T (head-cols on partitions) and V (tokens on partitions, with a
     constant ones-column appended at col 65) from streamed x^T tiles
  2. scores S^T[k,q] = K^T.T @ Q^T per head (K=64; two heads via PE row
     tiling on partitions 0-63 / 64-127, which run CONCURRENTLY); exp on
     ScalarE with the 1/8 softmax scale folded in; causal masking via gpsimd
     affine_select on diagonal blocks
  3. PV in [q, d] orientation: out[q, d] accumulates per 128-query chunk qc
     with lhsT = P^T[k, qc-block] and rhs = V[k, 65].  Matmul cost on the PE
     is output-free-size (65) per instruction, so this form costs ~520 rows
     per key block instead of 1024 in the [d, q] form (LDWEIGHTS of the
     128x128 stationary hides under the previous matmul; measured 36ns
     median issue spacing).  The ones-column of V makes the softmax
     denominator l land in PSUM column 64 per q-partition.
     PSUM has_written semantics: start=True clears the accumulate bits for
     the WHOLE bank, so only the chunk's very first PV matmul per head-bank
     carries start=True; the other qc slots first-write with start=False
     (bits clear after the bank-wide reset -> overwrite, then accumulate).
  4. chunk epilogue AFTER the last PV block (a PSUM bank must not be read
     while the PE can still write it - bank collisions are fatal):
     per-head reciprocal of the l column (one DVE op spanning all qc slots,
     which also carries the RAW dep on the final PV write), then per-qc
     tensor_scalar_mul into a [128 q, 128 feat] bf16 tile DMAd straight into
     the AllToAll buffer in [token, feature] layout.  The RECEIVER side
     loads a2a_out with dma_start_transpose (DMA-crossbar transpose, off all
     compute engines) to get the [feature, token] lhsT the out projection
     needs.
  5. attention runs j-major so AllToAll j fires at the earliest point; a
     dummy warmup collective absorbs CC init; out-projection chunks 0/1 run
     as PE filler in the exp-bound late quarters, chunk 2 is reserved (via
     tile_wait_until scheduler hints, sim-time ~ 0.85*HW-time) to fill the
     AllToAll-3 window, chunk 3 tails.  a2a_out loads are issued from the SP
     queue with hints so their CC-completion waits cannot head-of-line block
     a busy queue.
  6. QKV work units are fed one-at-a-time between attention blocks so the PE
     never starves while ScalarE chews the exps (keeps the PE p-state ramped)
"""

import os
import numpy as np
import ml_dtypes

B, N, C, H, HD = 4, 2048, 1024, 16, 64
SCALE = HD ** -0.5
P = 128
CB = C // P               # 8 contraction blocks
NQC = N // 512            # 4 query chunks per batch
TOK_G = 1024              # tokens per core after A2A (output slice rows)
NHC = 2                   # heads per core
VC = NHC * HD             # 128 v columns per core

BF16 = ml_dtypes.bfloat16

_CACHE = {}


def _build_nc():
    import concourse.bass as bass
    import concourse.tile as tile
    from concourse import bacc, mybir
    from concourse.bass import ts, ds
    from contextlib import ExitStack

    FP = mybir.dt.float32
    BF = mybir.dt.bfloat16
    EXP = mybir.ActivationFunctionType.Exp

    nc = bacc.Bacc(num_devices=8)

    # per-core inputs
    xT_p = nc.declare_dram_parameter("xT", [B, C, N], BF, isOutput=False)
    wqk_p = nc.declare_dram_parameter("wqk", [C, 2 * VC], BF, isOutput=False)
    wv_p = nc.declare_dram_parameter("wv", [C, VC], BF, isOutput=False)
    wout_p = nc.declare_dram_parameter("wout", [C, C], BF, isOutput=False)
    bqk_p = nc.declare_dram_parameter("bqk", [P, 2], FP, isOutput=False)
    bv_p = nc.declare_dram_parameter("bv", [1, VC], BF, isOutput=False)
    bout_p = nc.declare_dram_parameter("bout", [1, C], BF, isOutput=False)
    out_p = nc.declare_dram_parameter("out", [TOK_G, C], FP, isOutput=True)

    # A2A j carries, for target core s, the 256-token chunk
    # (batch s//2, queries [512j + 256*(s%2), +256)) in [token, feature]
    # layout; the receiver transposes on load via the DMA crossbar.
    a2a_in = [nc.dram_tensor(f"a2a_in{j}", [8, 256, P], BF) for j in range(NQC)]
    a2a_out = [nc.dram_tensor(f"a2a_out{j}", [8, 256, P], BF) for j in range(NQC)]
    warm_in = nc.dram_tensor("warm_in", [8, 1, 16], BF)
    warm_out = nc.dram_tensor("warm_out", [8, 1, 16], BF)

    def all_to_all(ins, outs):
        nc.gpsimd.collective_compute(
            "AllToAll",
            mybir.AluOpType.bypass,
            replica_groups=[list(range(8))],
            ins=[ins[:].opt()],
            outs=[outs[:].opt()],
        )

    with tile.TileContext(nc) as tc, ExitStack() as ctx:
        # dummy collective first: absorbs CC rendezvous + warmup latency
        all_to_all(warm_in, warm_out)

        const = ctx.enter_context(tc.tile_pool(name="const", bufs=1))
        big = ctx.enter_context(tc.tile_pool(name="big", bufs=1))
        xt_pool = ctx.enter_context(tc.tile_pool(name="xtp", bufs=4))
        pt_pool = ctx.enter_context(tc.tile_pool(name="ptp", bufs=5))
        # PSUM budget (16KB/partition): ps 3x[128,2,512]f32 = 12KB,
        # po 2x[128,4,128]f32 = 4KB (one bank per head, single chunk in
        # flight - the epilogue reads finish ~2us before the next chunk's
        # first PV needs the banks, and the deeper ps ring lets scores run
        # 3 blocks ahead of the exp stream instead of 2).
        ps_pool = ctx.enter_context(tc.tile_pool(name="psp", bufs=3, space="PSUM"))
        po_pool = ctx.enter_context(tc.tile_pool(name="pop", bufs=2, space="PSUM"))
        misc = ctx.enter_context(tc.tile_pool(name="misc", bufs=4))
        outp = ctx.enter_context(tc.tile_pool(name="outp", bufs=4))

        # first x tile and Q/K weights split in halves so the transfers run
        # in parallel across DMA queues and the PE can start ASAP
        xt_cache = {}

        def xt_fetch(b, tch, split=False):
            xt = xt_cache.get((b, tch))
            if xt is None:
                xt = xt_pool.tile([P, CB, 512], BF, tag="xt", name=f"xt{b}_{tch}")
                src = xT_p[b, :, ts(tch, 512)]
                if split:
                    nc.sync.dma_start(
                        xt[:, 0 : CB // 2, :],
                        src[0 : C // 2].rearrange("(cb p) t -> p cb t", p=P),
                    )
                    nc.sync.dma_start(
                        xt[:, CB // 2 : CB, :],
                        src[C // 2 : C].rearrange("(cb p) t -> p cb t", p=P),
                    )
                else:
                    nc.sync.dma_start(xt, src.rearrange("(cb p) t -> p cb t", p=P))
                xt_cache[(b, tch)] = xt
            return xt

        # first-half weights + first-half x first: the opening Q-chain's
        # kb=0..3 LDW/matmuls only touch those sub-regions, so the PE can
        # start while the second halves are still in flight
        wqk = const.tile([P, CB, 2 * VC], BF)
        nc.sync.dma_start(
            wqk[:, 0 : CB // 2, :],
            wqk_p[0 : C // 2].rearrange("(cb p) c -> p cb c", p=P),
        )
        xt_fetch(0, 0, split=True)
        nc.sync.dma_start(
            wqk[:, CB // 2 : CB, :],
            wqk_p[C // 2 : C].rearrange("(cb p) c -> p cb c", p=P),
        )
        wv = const.tile([P, CB, VC], BF)
        nc.sync.dma_start(wv, wv_p.rearrange("(cb p) c -> p cb c", p=P))
        bqk = const.tile([P, 2], FP)
        nc.sync.dma_start(bqk, bqk_p[:])
        bv = const.tile([1, VC], BF)
        nc.sync.dma_start(bv, bv_p[:])
        bvb = const.tile([P, VC], BF)
        nc.gpsimd.partition_broadcast(bvb, bv)

        # out-projection weights/bias tiles (DMA deferred past startup so the
        # 2MB transfer does not delay the first x-tile loads)
        wout = big.tile([P, CB, C], BF)
        bout = const.tile([1, C], BF)
        boutb = const.tile([P, C], BF)

        def load_wout():
            nc.sync.dma_start(wout, wout_p.rearrange("(cb p) c -> p cb c", p=P))
            nc.sync.dma_start(bout, bout_p[:])
            nc.gpsimd.partition_broadcast(boutb, bout)

        # per-(batch, 512-token-chunk) Q^T/K^T and V tiles for fine deps
        qk_t = [
            [big.tile([P, 2, 512], BF, name=f"qkT{b}_{t}") for t in range(NQC)]
            for b in range(B)
        ]
        v_t = []
        for b in range(B):
            row = []
            for t in range(NQC):
                vt = big.tile([P, 4, NHC, HD + 1], BF, name=f"v{b}_{t}")
                nc.vector.memset(vt[:, :, :, HD : HD + 1], 1.0)
                row.append(vt)
            v_t.append(row)

        # ---- QKV work units ----------------------------------------------
        # unit order per group (b,t): [Q, K, V0, V1, V2, V3]; every unit
        # closes its own PSUM tile so pool rotation can never deadlock the
        # in-order PE behind a later-emitted consumer
        def prefetch_ahead():
            # keep up to 3 not-yet-consumed x tiles in flight so no group's
            # first matmul ever waits on its 1MB DMA (a one-ahead prefetch
            # misses the demand jumps at phase transitions: 19us stall)
            pending = sum(
                1 for g in feed_seq if g in xt_cache and emitted[g] == 0
            )
            if pending >= 3:
                return
            for g in feed_seq:
                if emitted[g] < 6 and g not in xt_cache:
                    xt_fetch(*g)
                    return

        def qkv_unit(b, tch, u):
            xt = xt_fetch(b, tch)
            if u == 0:
                prefetch_ahead()
            if u < 2:
                qk = u
                psq = ps_pool.tile([P, 512], FP, tag="ps", name=f"psq{b}_{tch}_{qk}")
                for kb in range(CB):
                    nc.tensor.matmul(
                        psq,
                        lhsT=wqk[:, kb, ts(qk, P)],
                        rhs=xt[:, kb, :],
                        start=(kb == 0),
                        stop=(kb == CB - 1),
                        skip_group_check=True,
                    )
                nc.vector.tensor_scalar_add(
                    qk_t[b][tch][:, qk, :], psq, bqk[:, qk : qk + 1]
                )
            else:
                tb4 = u - 2
                psv = ps_pool.tile([P, VC], FP, tag="ps", name=f"psv{b}_{tch}_{tb4}")
                for kb in range(CB):
                    nc.tensor.matmul(
                        psv,
                        lhsT=xt[:, kb, ts(tb4, P)],
                        rhs=wv[:, kb, :],
                        start=(kb == 0),
                        stop=(kb == CB - 1),
                        skip_group_check=True,
                    )
                nc.vector.tensor_tensor(
                    v_t[b][tch][:, tb4, :, 0:HD],
                    psv.rearrange("p (h d) -> p h d", h=NHC),
                    bvb.rearrange("p (h d) -> p h d", h=NHC),
                    mybir.AluOpType.add,
                )

        # feed machinery: units emitted on demand (deps) or popped as filler
        emitted = {(b, t): 0 for b in range(B) for t in range(NQC)}
        feed_seq = [(b, t) for t in range(NQC) for b in range(B)]

        def emit_group(b, t, upto):
            while emitted[(b, t)] < upto:
                qkv_unit(b, t, emitted[(b, t)])
                emitted[(b, t)] += 1

        def pop_units(n, tmax=NQC):
            while n > 0:
                prefetch_ahead()
                for g in feed_seq:
                    if emitted[g] < 6:
                        qkv_unit(g[0], g[1], emitted[g])
                        emitted[g] += 1
                        break
                else:
                    return
                n -= 1

        # ---- attention ----------------------------------------------------
        def emit_attention(b, j):
            emit_group(b, j, 2)  # biased Q (and K) of chunk j
            # per-head PV accumulators: one PSUM bank each;
            # [q-in-chunk partitions, qc, 65 used of 128]
            po = [
                po_pool.tile([P, 4, P], FP, tag="po", name=f"po{b}_{j}_{hh}")
                for hh in range(2)
            ]
            nkb = 4 * j + 4
            pts = {}

            def emit_scores(i):
                m = max(0, i - 4 * j)
                w = 512 - P * m
                emit_group(b, i // 4, 2)           # K of this key chunk
                emit_group(b, i // 4, 3 + i % 4)   # V block for this i
                i2 = min(i + 2, nkb - 1)           # V two blocks ahead so the
                emit_group(b, i2 // 4, 3 + i2 % 4)  # fp PV never waits the DVE
                pss = ps_pool.tile([P, 2, 512], FP, tag="ps", name=f"pss{b}_{j}_{i}")
                for hh in range(2):
                    rlo = 64 * hh
                    nc.tensor.matmul(
                        pss[:, hh, P * m : 512],
                        lhsT=qk_t[b][i // 4][:, 1, :][rlo : rlo + 64, ts(i % 4, P)],
                        rhs=qk_t[b][j][:, 0, :][rlo : rlo + 64, ds(P * m, w)],
                        start=True,
                        stop=True,
                        skip_group_check=True,
                    )
                pt = pt_pool.tile([P, 2, 512], BF, tag="pt", name=f"pt{b}_{j}_{i}")
                nc.scalar.activation(
                    pt[:, :, P * m : 512], pss[:, :, P * m : 512], EXP, scale=SCALE
                )
                if i >= 4 * j:
                    nc.gpsimd.affine_select(
                        out=pt[:, :, P * m : P * m + P],
                        in_=pt[:, :, P * m : P * m + P],
                        compare_op=mybir.AluOpType.is_ge,
                        fill=0.0,
                        base=0,
                        pattern=[[0, 2], [1, P]],
                        channel_multiplier=-1,
                    )
                pts[i] = pt

            def emit_pv(i):
                m = max(0, i - 4 * j)
                pt = pts.pop(i)
                for hh in range(2):
                    for qc in range(m, 4):
                        nc.tensor.matmul(
                            po[hh][:, qc, 0 : HD + 1],
                            lhsT=pt[:, hh, ts(qc, P)],
                            rhs=v_t[b][i // 4][:, i % 4, hh, :],
                            start=(i == 0 and qc == 0),
                            stop=(i == 4 * j + qc),
                            skip_group_check=True,
                        )

            # 2-block software pipeline skew with filler popped before the
            # PV so exp(i)/mask(i) are long done when PV(i) issues
            for i in range(nkb):
                emit_scores(i)
                if i >= 2:
                    emit_pv(i - 2)
                pop_units(1)
            emit_pv(nkb - 2)
            emit_pv(nkb - 1)

            # epilogue (after ALL PV writes into the po banks): normalize by
            # the l rider column and stage into the A2A buffer [tok, feat].
            # The rec read spans all qc slots so it carries the dep on the
            # bank's final PV write; the muls follow in DVE program order.
            rec = misc.tile([P, 2, 4, 1], FP, tag="rec")
            for hh in range(2):
                nc.vector.reciprocal_approx_fast(
                    rec[:, hh], po[hh][:, :, HD : HD + 1]
                )
            for qc in range(4):
                at_qd = misc.tile([P, 2, HD], BF, tag="atqd")
                for hh in range(2):
                    nc.vector.tensor_scalar_mul(
                        at_qd[:, hh, :], po[hh][:, qc, 0:HD], rec[:, hh, qc, :]
                    )
                nc.sync.dma_start(
                    a2a_in[j][2 * b + qc // 2][ds(P * (qc % 2), P), :], at_qd
                )

        # ---- out projection for one 256-token chunk ----------------------
        at_all = [[None] * CB for _ in range(NQC)]
        # tile_wait_until hints (scheduler-sim ms ~ 0.85e-3 * HW ns): keep
        # the CC-gated loads from being scheduled ahead of satisfiable work
        load_hint = {0: 0.120, 1: 0.190, 2: 0.245, 3: 0.290}

        def outproj_load(k):
            with tc.tile_wait_until(load_hint[k]):
                for kb in range(CB):
                    t = big.tile([P, 256], BF, name=f"at_all{k}_{kb}")
                    nc.sync.dma_start_transpose(t, a2a_out[k][kb])
                    at_all[k][kb] = t

        def outproj_chain(k, tb, co):
            if at_all[k][0] is None:
                outproj_load(k)
            py = ps_pool.tile([P, 512], FP, tag="ps")
            for kb in range(CB):
                nc.tensor.matmul(
                    py,
                    lhsT=at_all[k][kb][:, ts(tb, P)],
                    rhs=wout[:, kb, ts(co, 512)],
                    start=(kb == 0),
                    stop=(kb == CB - 1),
                    skip_group_check=True,
                )
            ot = outp.tile([P, 512], FP, tag="ot")
            nc.vector.tensor_add(ot, py, boutb[:, ts(co, 512)])
            nc.sync.dma_start(out_p[ds(256 * k + P * tb, P), ts(co, 512)], ot)

        def outproj_chunk(k, chain_hint=None):
            if at_all[k][0] is None:
                outproj_load(k)
            from contextlib import nullcontext

            cm = tc.tile_wait_until(chain_hint) if chain_hint else nullcontext()
            with cm:
                for tb in range(2):
                    for co in range(2):
                        outproj_chain(k, tb, co)

        # ---- main schedule ------------------------------------------------
        # out-projection chunks 0/1 double as PE filler for the exp-bound
        # late quarters; chunk 2 is held back (hint) to fill the AllToAll-3
        # window; chunk 3 tails.
        placements = {(2, 0): 0, (3, 0): 1}
        for j in range(NQC):
            for b in range(B):
                emit_attention(b, j)
                k = placements.get((j, b))
                if k is not None:
                    outproj_chunk(k)
            all_to_all(a2a_in[j], a2a_out[j])
            if j == 0:
                load_wout()
        pop_units(9999)
        outproj_chunk(2, chain_hint=0.288)
        outproj_chunk(NQC - 1)

    nc.finalize()
    return nc


def _get_nc():
    if "nc" not in _CACHE:
        _CACHE["nc"] = _build_nc()
    return _CACHE["nc"]


def _shard_inputs(x, w_qkv, b_qkv, w_out, b_out):
    x = np.asarray(x, dtype=np.float32)
    w_qkv = np.asarray(w_qkv, dtype=np.float32)
    b_qkv = np.asarray(b_qkv, dtype=np.float32)
    w_out = np.asarray(w_out, dtype=np.float32)
    b_out = np.asarray(b_out, dtype=np.float32)

    xT = np.ascontiguousarray(x.transpose(0, 2, 1)).astype(BF16)  # (B, C, N)
    wout_b = np.ascontiguousarray(w_out).astype(BF16)
    bout_r = np.ascontiguousarray(b_out[None, :]).astype(BF16)

    in_maps = []
    for c in range(8):
        c0 = HD * NHC * c  # first head-col owned by this core
        wq = w_qkv[:, 0 * C + c0 : 0 * C + c0 + VC]
        wk = w_qkv[:, 1 * C + c0 : 1 * C + c0 + VC]
        wvv = w_qkv[:, 2 * C + c0 : 2 * C + c0 + VC]
        bq = b_qkv[0 * C + c0 : 0 * C + c0 + VC]
        bk = b_qkv[1 * C + c0 : 1 * C + c0 + VC]
        bvv = b_qkv[2 * C + c0 : 2 * C + c0 + VC]
        in_maps.append(
            dict(
                xT=xT,
                wqk=np.ascontiguousarray(np.concatenate([wq, wk], axis=1)).astype(BF16),
                wv=np.ascontiguousarray(wvv).astype(BF16),
                wout=wout_b,
                bqk=np.ascontiguousarray(np.stack([bq, bk], axis=1)).astype(np.float32),
                bv=np.ascontiguousarray(bvv[None, :]).astype(BF16),
                bout=bout_r,
            )
        )
    return in_maps


def kernel(x, attention_mask, w_qkv, b_qkv, w_out, b_out):
    from concourse.bass_utils import run_bass_kernel_spmd

    nc = _get_nc()
    in_maps = _shard_inputs(x, w_qkv, b_qkv, w_out, b_out)
    res = run_bass_kernel_spmd(nc, in_maps, core_ids=list(range(8)))
    _CACHE["last_results"] = res
    out = np.empty((B, N, C), np.float32)
    for c in range(8):
        b = c // 2
        h = c % 2
        r = np.asarray(res.results[c]["out"]).astype(np.float32)
        for j in range(NQC):
            t0 = 512 * j + 256 * h
            out[b, t0 : t0 + 256] = r[256 * j : 256 * j + 256]
    return out
